# revision 20
# baseline (speedup 1.0000x reference)
"""Soft decision-tree forward (nn_DTree) on 8 trn2 NeuronCores.

Strategy (pure data parallel, per the sharding hint):
  - shard x row-wise 8 ways; replicate the tiny tree params.
  - per core: z = x @ W^T - c via bf16 PE matmuls into fp32 PSUM
    (the -c bias is a second accumulating matmul in PE row-group 1,
    K=2 ones against [-c_hi; -c_lo] split-precision rows),
    g = sigmoid(z) on ACT, then a level-by-level value-tree blend on DVE:
       V_k = g_k * (V_{k+1,L} - V_{k+1,R}) + V_{k+1,R}
    with nodes pre-permuted (level-major, left-children-first) so every
    level's children are two contiguous halves of the previous level.
  - x reaches the PE transposed: fp32->bf16 cast DMA into a DRAM bounce,
    then HWDGE xbar dma-transpose into SBUF as [32, rows].
"""

import numpy as np
import ml_dtypes

import concourse.bass as bass
import concourse.bacc as bacc
import concourse.tile as tile
from concourse import mybir
from concourse.bass_utils import run_bass_kernel_spmd

BF16 = ml_dtypes.bfloat16

F = 32
D = 8
NODES = 255
LEAVES = 256
N_FULL = 262144
N_CORES = 8
ROWS = N_FULL // N_CORES  # 32768 rows per core

# level-major offsets of each level's gates inside the 255-column block
LEVEL_OFF = {7: 0, 6: 128, 5: 192, 4: 224, 3: 240, 2: 248, 1: 252, 0: 254}


def _orderings():
    """ord[k] = local node order at level k (left-children-first recursion)."""
    ordv = {0: [0]}
    for k in range(7):
        ordv[k + 1] = [2 * i for i in ordv[k]] + [2 * i + 1 for i in ordv[k]]
    col_nodes = []
    for k in range(7, -1, -1):
        base = 2 ** k - 1
        col_nodes += [base + i for i in ordv[k]]
    return ordv, np.array(col_nodes)


def host_prep(feature_importances, feature_splits, leaf_node_classes, slots):
    """Tiny-param preprocessing (O(8K) work): relu/sigmoid/c, node permutation,
    bf16 weight matrix with split bias rows, leaf-blend constants."""
    fi = np.asarray(feature_importances, np.float32).reshape(NODES, F)
    fs = np.asarray(feature_splits, np.float32).reshape(NODES, F)
    cls = np.asarray(leaf_node_classes, np.float32).reshape(LEAVES)

    W = np.maximum(fi, 0.0)
    S = 1.0 / (1.0 + np.exp(-fs))
    c = np.sum(W * S, axis=1)  # (NODES,)

    ordv, col_nodes = _orderings()
    Wp = W[col_nodes]          # (255, 32) permuted level-major
    cp = c[col_nodes]

    c_hi = cp.astype(BF16).astype(np.float32)
    c_lo = (cp - c_hi).astype(np.float32)

    wt = np.zeros((34, 256), BF16)
    wt[0:F, 0:NODES] = Wp.T.astype(BF16)
    wt[F, 0:NODES] = (-c_hi).astype(BF16)
    wt[F + 1, 0:NODES] = (-c_lo).astype(BF16)

    o7 = np.array(ordv[7])
    delta = (cls[2 * o7] - cls[2 * o7 + 1]).astype(BF16)
    beta = cls[2 * o7 + 1].astype(BF16)
    db = np.zeros((128, 2 * slots * 128), BF16)
    db[:, : slots * 128] = np.tile(delta, slots)[None, :]
    db[:, slots * 128 :] = np.tile(beta, slots)[None, :]
    return wt, db


def build_nc(rows, slots, chunk, use_bias_mm=True, host_xt=False, simple_out=False):
    """Build the single-core Bass program (SPMD across the cores)."""
    assert rows % 128 == 0 and chunk % 128 == 0 and rows % chunk == 0
    tiles = rows // 128
    assert tiles % slots == 0
    groups = tiles // slots
    bf = mybir.dt.bfloat16
    f32 = mybir.dt.float32

    nc = bacc.Bacc()
    if host_xt:
        xt_in = nc.dram_tensor("xthost", [F, rows], bf, kind="ExternalInput")
    else:
        x_in = nc.dram_tensor("x", [rows, F], f32, kind="ExternalInput")
    wt_in = nc.dram_tensor("wt", [34, 256], bf, kind="ExternalInput")
    db_in = nc.dram_tensor("db", [128, 2 * slots * 128], bf, kind="ExternalInput")
    ones_in = nc.dram_tensor("ones", [2, rows], bf, kind="ExternalInput")
    if simple_out:
        out_dram = nc.dram_tensor("out", [128, rows // 128], f32, kind="ExternalOutput")
    else:
        out_dram = nc.dram_tensor("out", [rows, 1], f32, kind="ExternalOutput")

    with tile.TileContext(nc) as tc:
        with (
            tc.tile_pool(name="consts", bufs=1) as consts,
            tc.tile_pool(name="xT", bufs=1) as xtp,
            tc.tile_pool(name="dram", bufs=1, space="DRAM") as dram,
            tc.tile_pool(name="zps", bufs=2, space="PSUM") as zps,
            tc.tile_pool(name="gpool", bufs=2) as gpool,
            tc.tile_pool(name="vpool", bufs=2) as vpool,
            tc.tile_pool(name="dpool", bufs=2) as dpool,
            tc.tile_pool(name="opool", bufs=1) as opool,
        ):
            # ---- constants (each SBUF region has exactly one producer) ----
            wt_sb = consts.tile([34, 256], bf)
            nc.sync.dma_start(out=wt_sb[:], in_=wt_in[:])
            dbt = consts.tile([128, 2 * slots * 128], bf)
            nc.sync.dma_start(out=dbt[:], in_=db_in[:])
            dbc = dbt[:, 0 : slots * 128].rearrange("p (s c) -> p s c", c=128)
            bbc = dbt[:, slots * 128 :].rearrange("p (s c) -> p s c", c=128)
            # ---- x: cast to bf16 in DRAM, xbar-transpose into SBUF.
            # Rows 32-33 of xT are constant 1.0 (bias feature for -c_hi/-c_lo).
            xT = xtp.tile([F + 2, rows], bf)
            nc.sync.dma_start(out=xT[F : F + 2, :], in_=ones_in[:])
            if host_xt:
                for ci in range(rows // chunk):
                    sl = slice(ci * chunk, (ci + 1) * chunk)
                    nc.sync.dma_start(out=xT[0:F, sl], in_=xt_in[:, sl])
            else:
                xbf = dram.tile([rows, F], bf)
                for ci in range(rows // chunk):
                    sl = slice(ci * chunk, (ci + 1) * chunk)
                    nc.gpsimd.dma_start(out=xbf[sl, :], in_=x_in[sl, :])
                    nc.sync.dma_start_transpose(out=xT[0:F, sl], in_=xbf[sl, :])

            out_sb = opool.tile([128, tiles], f32)

            pw = min(8, slots)  # tiles per PSUM wave
            for gi in range(groups):
                g_t = gpool.tile([128, slots, 256], bf)
                for half in range(slots // pw):
                    zt = zps.tile([128, pw * 256], f32)
                    ztv = zt[:].rearrange("p (j c) -> p j c", c=256)
                    kk = F + 2 if use_bias_mm else F
                    for j in range(pw):
                        t = gi * slots + half * pw + j
                        nc.tensor.matmul(
                            ztv[:, j, 0:NODES],
                            lhsT=xT[0:kk, t * 128 : (t + 1) * 128],
                            rhs=wt_sb[0:kk, 0:NODES],
                            start=True,
                            stop=True,
                        )
                    nc.scalar.activation(
                        out=g_t[:, half * pw : (half + 1) * pw, 0:NODES],
                        in_=ztv[:, :, 0:NODES],
                        func=mybir.ActivationFunctionType.Sigmoid,
                    )
                # ---- value tree ----
                v = vpool.tile([128, slots, 128], bf, tag="v7")
                nc.vector.tensor_mul(v[:], g_t[:, :, 0:128], dbc)
                nc.vector.tensor_add(v[:], v[:], bbc)
                for k in range(6, -1, -1):
                    m = 2 ** k
                    off = LEVEL_OFF[k]
                    vl = v[:, :, 0:m]
                    vr = v[:, :, m : 2 * m]
                    dt = dpool.tile([128, slots, m], bf, tag="dtmp")
                    nc.vector.tensor_sub(dt[:], vl, vr)
                    if k > 0:
                        vn = vpool.tile([128, slots, m], bf, tag=f"v{k}")
                        nc.vector.tensor_mul(vn[:], g_t[:, :, off : off + m], dt[:])
                        nc.vector.tensor_add(vn[:], vn[:], vr)
                        v = vn
                    else:
                        vo = out_sb[:, gi * slots : (gi + 1) * slots]
                        vov = vo.rearrange("p (s o) -> p s o", o=1)
                        nc.vector.tensor_mul(vov, g_t[:, :, off : off + m], dt[:])
                        nc.vector.tensor_add(vov, vov, vr)

            # ---- store [128, tiles] -> out[t*128 + p] (strided) ----
            if simple_out:
                nc.sync.dma_start(out=out_dram[:], in_=out_sb[:])
            else:
                ov = out_dram[:].rearrange("(t p) c -> p (t c)", p=128)
                nc.sync.dma_start(out=ov, in_=out_sb[:])
    return nc


_CACHE = {}


def _get_nc(rows, slots, chunk):
    key = (rows, slots, chunk)
    if key not in _CACHE:
        nc = build_nc(rows, slots, chunk)
        if not nc.is_finalized():
            nc.finalize()
        _CACHE[key] = nc
    return _CACHE[key]


def run_device(x, wt, db, slots, chunk, n_cores=N_CORES, trace=False):
    rows = x.shape[0] // n_cores
    nc = _get_nc(rows, slots, chunk)
    ones1 = np.ones((2, rows), BF16)
    in_maps = [
        {
            "x": np.ascontiguousarray(x[i * rows : (i + 1) * rows]),
            "wt": wt,
            "db": db,
            "ones": ones1,
        }
        for i in range(n_cores)
    ]
    res = run_bass_kernel_spmd(nc, in_maps, list(range(n_cores)), trace=trace)
    out = np.concatenate([res.results[i]["out"] for i in range(n_cores)], axis=0)
    return out.astype(np.float32), res


def kernel(**inputs):
    x = np.asarray(inputs["x"], np.float32).reshape(-1, F)
    slots, chunk = 32, 8192
    wt, db = host_prep(
        inputs["feature_importances"],
        inputs["feature_splits"],
        inputs["leaf_node_classes"],
        slots,
    )
    out, _ = run_device(x, wt, db, slots, chunk)
    return out
